# revision 1
# baseline (speedup 1.0000x reference)
"""Ball-query + grouping kernel builder for one NeuronCore (SPMD over 8).

Per-core: pts [3,16384], ctr [3,1024], feat [64,16384] -> out [67,1024,32].
out[0:3] = gathered coords - center, out[3:67] = gathered features, where
gather indices = first 32 point indices with dist^2 < 0.01 per center
(ascending, padded with first index / 0).

Also takes host-precomputed index-pattern constants:
  lut32  [128, 2048] int32: lut[v*8+r] = position of (r+1)-th set bit of byte v
  lutsel [128, 4096] int16: (c%16 == p%16) ? c//16 : -1   (diagonal extract)
  wio16  [128, 2048] int16: word index + 1
  kio256 [128, 256] int32: k % 32

Pipeline (per 128-center tile):
  PE:  v = r2 - d2 via bf16 3-split matmul, 30 contraction rows ordered so
       partial sums cancel early (hi | mid | lo groups)
  ACT: s = Sign(v - eps)
  DVE: red[w] = sum_8 s*(256+2^b) -> U = 256*cnt8 + packed8 (per 8-pt word)
  DVE: I16 = scan(cnt8); E16 = shift(I16); sidx = min(E16, 33)
  GPS: local_scatter (dup-tolerant, last-write-wins) word-id and
       E16*256+packed8 into 34 rank slots; per-tile prefix-max fill
Batched tail (all 8 tiles as [128, 256]):
  DVE smalls -> lutidx; GPS ap_gather LUT + local_scatter diagonal extract
  -> bit position; final index assembly + select; wrap; ap_gather of
  [feat;coords]; coord subtract; DMA out.
"""

import numpy as np
import concourse.bass as bass
import concourse.mybir as mybir
from concourse import bacc
from concourse.tile import TileContext

dt = mybir.dt
Alu = mybir.AluOpType
AFT = mybir.ActivationFunctionType

N = 16384
MH = 1024
C = 64
K = 32
NT = MH // 128
NW = N // 8
CH = 512
NCH = N // CH
R2 = np.float32(np.float64(0.1) * np.float64(0.1))
W8SUM = float(sum((256 + (1 << b)) for b in range(8)))  # 2303


def host_consts():
    wio16 = np.broadcast_to(np.arange(1, NW + 1, dtype=np.int16), (128, NW)).copy()
    kio256 = np.broadcast_to((np.arange(256) % 32).astype(np.int32), (128, 256)).copy()
    return {"wio16": wio16, "kio256": kio256}


def _split3_dev(nc, pool, src_f32, tag):
    shape = list(src_f32.shape)
    hi = pool.tile(shape, dt.bfloat16, tag=f"{tag}_hi")
    mid = pool.tile(shape, dt.bfloat16, tag=f"{tag}_mid")
    lo = pool.tile(shape, dt.bfloat16, tag=f"{tag}_lo")
    r1 = pool.tile(shape, dt.float32, tag=f"{tag}_r1")
    r2 = pool.tile(shape, dt.float32, tag=f"{tag}_r2")
    nc.vector.tensor_copy(hi[:], src_f32[:])
    nc.vector.tensor_tensor(out=r1[:], in0=src_f32[:], in1=hi[:], op=Alu.subtract)
    nc.vector.tensor_copy(mid[:], r1[:])
    nc.vector.tensor_tensor(out=r2[:], in0=r1[:], in1=mid[:], op=Alu.subtract)
    nc.vector.tensor_copy(lo[:], r2[:])
    return hi, mid, lo


def build(debug=False):
    nc = bacc.Bacc("TRN2", target_bir_lowering=False, debug=False, num_devices=8)
    pts = nc.dram_tensor("pts", [3, N], dt.float32, kind="ExternalInput")
    ctr = nc.dram_tensor("ctr", [3, MH], dt.float32, kind="ExternalInput")
    feat = nc.dram_tensor("feat", [C, N], dt.float32, kind="ExternalInput")
    wio16_d = nc.dram_tensor("wio16", [128, NW], dt.int16, kind="ExternalInput")
    kio256_d = nc.dram_tensor("kio256", [128, 256], dt.int32, kind="ExternalInput")
    out = nc.dram_tensor("out", [3 + C, MH, K], dt.float32, kind="ExternalOutput")
    dbg = {}
    if debug:
        dbg["idx"] = nc.dram_tensor("dbg_idx", [MH, K], dt.int32, kind="ExternalOutput")

    with TileContext(nc) as tc:
        pers_cm = tc.tile_pool(name="pers", bufs=1)
        pers = pers_cm.__enter__()
        with tc.tile_pool(name="prep", bufs=1) as prep:
            # ---------- persistent inputs ----------
            bigbuf = pers.tile([128, N], dt.float32)   # rows 0..63 feat, 64..66 pts
            nc.sync.dma_start(bigbuf[0:64, :], feat[:, :])
            nc.sync.dma_start(bigbuf[64:67, :], pts[:, :])
            crow = pers.tile([128, MH], dt.float32)
            nc.sync.dma_start(crow[64:67, :], ctr[:, :])
            wio = pers.tile([128, NW], dt.int16)
            nc.sync.dma_start(wio[:], wio16_d[:, :])
            kio = pers.tile([128, 256], dt.int32)
            nc.sync.dma_start(kio[:], kio256_d[:, :])

            # ---------- build RHS [30, N] bf16 ----------
            # rows: 0-2 ph, 3 q2h, 4 ones, 5-7 pm, 8-10 ph, 11 q2m, 12 ones,
            #       13-15 pl, 16-18 ph, 19-21 pm, 22-24 pl, 25-27 pm, 28 q2l, 29 ones
            RHS = pers.tile([30, N], dt.bfloat16)
            nc.vector.memset(RHS[:, :], 1.0)
            pv = prep.tile([96, 512], dt.float32)
            nc.sync.dma_start(pv[:], pts.rearrange("d (a b) -> (d a) b", b=512))
            ph, pm, pl = _split3_dev(nc, prep, pv, "p")
            for piece, rows in ((ph, (0, 8, 16)), (pm, (5, 19, 25)), (pl, (13, 22))):
                r0 = rows[0]
                nc.sync.dma_start(
                    RHS[r0:r0 + 3, :].rearrange("d (a b) -> d a b", b=512), piece[:]
                )
                for r in rows[1:]:
                    nc.sync.dma_start(RHS[r:r + 3, :], RHS[r0:r0 + 3, :])
            qv = prep.tile([96, 512], dt.float32)
            nc.vector.tensor_tensor(out=qv[:], in0=pv[:], in1=pv[:], op=Alu.mult)
            qy = prep.tile([32, 512], dt.float32)
            qz = prep.tile([32, 512], dt.float32)
            nc.sync.dma_start(qy[:], qv[32:64, :])
            nc.sync.dma_start(qz[:], qv[64:96, :])
            q2 = prep.tile([32, 512], dt.float32)
            nc.vector.tensor_tensor(out=q2[:], in0=qv[0:32, :], in1=qy[:], op=Alu.add)
            nc.vector.tensor_tensor(out=q2[:], in0=q2[:], in1=qz[:], op=Alu.add)
            q2h, q2m, q2l = _split3_dev(nc, prep, q2, "q2")
            for piece, r in ((q2h, 3), (q2m, 11), (q2l, 28)):
                nc.sync.dma_start(
                    RHS[r:r + 1, :].rearrange("e (a b) -> e a b", b=512), piece[:]
                )

            # ---------- build CAUG [30, MH] bf16 ----------
            # rows: 0-2 2ch, 3 -1, 4 wh, 5-7 2ch, 8-10 2cm, 11 -1, 12 wm,
            #       13-15 2ch, 16-18 2cl, 19-21 2cm, 22-24 2cm, 25-27 2cl, 28 -1, 29 wl
            CAUG = pers.tile([30, MH], dt.bfloat16)
            nc.vector.memset(CAUG[:, :], -1.0)
            cv = prep.tile([96, 32], dt.float32)
            nc.sync.dma_start(cv[:], ctr.rearrange("d (a b) -> (d a) b", b=32))
            chs, cms, cls = _split3_dev(nc, prep, cv, "c")
            ch2 = prep.tile([96, 32], dt.bfloat16)
            cm2 = prep.tile([96, 32], dt.bfloat16)
            cl2 = prep.tile([96, 32], dt.bfloat16)
            nc.vector.tensor_scalar(out=ch2[:], in0=chs[:], scalar1=2.0, scalar2=None, op0=Alu.mult)
            nc.vector.tensor_scalar(out=cm2[:], in0=cms[:], scalar1=2.0, scalar2=None, op0=Alu.mult)
            nc.vector.tensor_scalar(out=cl2[:], in0=cls[:], scalar1=2.0, scalar2=None, op0=Alu.mult)
            for piece, rows in ((ch2, (0, 5, 13)), (cm2, (8, 19, 22)), (cl2, (16, 25))):
                r0 = rows[0]
                nc.sync.dma_start(
                    CAUG[r0:r0 + 3, :].rearrange("d (a b) -> d a b", b=32), piece[:]
                )
                for r in rows[1:]:
                    nc.sync.dma_start(CAUG[r:r + 3, :], CAUG[r0:r0 + 3, :])
            cq = prep.tile([96, 32], dt.float32)
            nc.vector.tensor_tensor(out=cq[:], in0=cv[:], in1=cv[:], op=Alu.mult)
            cqy = prep.tile([32, 32], dt.float32)
            cqz = prep.tile([32, 32], dt.float32)
            nc.sync.dma_start(cqy[:], cq[32:64, :])
            nc.sync.dma_start(cqz[:], cq[64:96, :])
            c2t = prep.tile([32, 32], dt.float32)
            nc.vector.tensor_tensor(out=c2t[:], in0=cq[0:32, :], in1=cqy[:], op=Alu.add)
            nc.vector.tensor_tensor(out=c2t[:], in0=c2t[:], in1=cqz[:], op=Alu.add)
            wt = prep.tile([32, 32], dt.float32)
            nc.vector.tensor_scalar(
                out=wt[:], in0=c2t[:], scalar1=-1.0, scalar2=float(R2), op0=Alu.mult, op1=Alu.add
            )
            wh, wm, wl = _split3_dev(nc, prep, wt, "w")
            for piece, r in ((wh, 4), (wm, 12), (wl, 29)):
                nc.sync.dma_start(
                    CAUG[r:r + 1, :].rearrange("e (a b) -> e a b", b=32), piece[:]
                )

            # ---------- constants ----------
            w8 = pers.tile([128, 8], dt.float16)
            for b in range(8):
                nc.vector.memset(w8[:, b:b + 1], float(256 + (1 << b)))
            negeps = pers.tile([128, 1], dt.float32)
            nc.vector.memset(negeps[:], -1e-12)

        Wall = pers.tile([128, 256], dt.float32)
        Dall = pers.tile([128, 256], dt.float32)
        ctall = pers.tile([128, 8], dt.float32)
        i16o = pers.tile([128, 256], dt.int16)
        wrp = pers.tile([16, 2048], dt.int16)
        wrep = pers.tile([128, 2048], dt.int16)

        with (
            tc.tile_pool(name="sgrp", bufs=2) as sgrp_pool,
            tc.tile_pool(name="s2p", bufs=1) as s2p,
            tc.tile_pool(name="redp", bufs=1) as redp,
            tc.tile_pool(name="nwp", bufs=1) as nwp,
            tc.tile_pool(name="sml", bufs=1) as sml,
            tc.tile_pool(name="psum", bufs=4, space="PSUM") as psum,
            tc.tile_pool(name="gat", bufs=1) as gat,
        ):
            GRP = 4
            NG = NCH // GRP
            for t in range(NT):
                lhsT = CAUG[:, t * 128:(t + 1) * 128]
                red = redp.tile([128, NW], dt.float32, tag="red")
                for g in range(NG):
                    S = sgrp_pool.tile([128, GRP * CH], dt.bfloat16, tag="S")
                    for ci in range(GRP):
                        ch = g * GRP + ci
                        vp = psum.tile([128, CH], dt.float32, tag="v")
                        nc.tensor.matmul(
                            vp[:], lhsT, RHS[:, ch * CH:(ch + 1) * CH],
                            start=True, stop=True,
                        )
                        nc.scalar.activation(
                            S[:, ci * CH:(ci + 1) * CH], vp[:], AFT.Sign,
                            bias=negeps[:], scale=1.0,
                        )
                    S2 = s2p.tile([128, GRP * CH], dt.float16, tag="S2")
                    nc.vector.tensor_tensor(
                        out=S2[:],
                        in0=S[:],
                        in1=w8[:].unsqueeze(1).to_broadcast([128, GRP * CH // 8, 8]),
                        op=Alu.mult,
                    )
                    nc.vector.tensor_reduce(
                        out=red[:, g * (GRP * CH // 8):(g + 1) * (GRP * CH // 8)],
                        in_=S2[:].rearrange("p (w e) -> p w e", e=8),
                        axis=mybir.AxisListType.X,
                        op=Alu.add,
                    )
                # red = 2*(256*cnt8 + packed8) - 2303
                U = nwp.tile([128, NW], dt.int32, tag="U")
                nc.vector.tensor_scalar(
                    out=U[:], in0=red[:], scalar1=W8SUM, scalar2=0.5,
                    op0=Alu.add, op1=Alu.mult,
                )
                cnt8i = nwp.tile([128, NW], dt.int32, tag="cnt8i")
                nc.vector.tensor_scalar(
                    out=cnt8i[:], in0=U[:], scalar1=8, scalar2=None,
                    op0=Alu.logical_shift_right,
                )
                cnt8b = nwp.tile([128, NW], dt.bfloat16, tag="cnt8b")
                nc.vector.tensor_copy(cnt8b[:], cnt8i[:])
                # p8 = U & 255 in place (after cnt8b read)
                nc.vector.tensor_scalar(
                    out=U[:], in0=U[:], scalar1=255, scalar2=None, op0=Alu.bitwise_and,
                )
                I16 = nwp.tile([128, NW], dt.bfloat16, tag="I16")
                nc.vector.tensor_tensor_scan(
                    out=I16[:], data0=cnt8b[:], data1=cnt8b[:],
                    initial=0.0, op0=Alu.add, op1=Alu.max,
                )
                nc.vector.tensor_copy(ctall[:, t:t + 1], I16[:, NW - 1:NW])
                # E16 (exclusive prefix) = I16 shifted by one word, read
                # via offset APs. Only non-empty words scatter; empty -> -1.
                # E16 >= 32 clamps to slot 33 (trash; dups possible there but
                # those slots are never read).
                acl = nwp.tile([128, NW], dt.bfloat16, tag="acl")
                nc.vector.memset(acl[:, 0:1], 0.0)
                nc.vector.tensor_scalar(out=acl[:, 1:NW], in0=I16[:, 0:NW - 1], scalar1=32.0, scalar2=None, op0=Alu.min)
                mg = nwp.tile([128, NW], dt.bfloat16, tag="mg")
                nc.vector.tensor_scalar(out=mg[:], in0=cnt8b[:], scalar1=0.0, scalar2=None, op0=Alu.is_gt)
                u1 = nwp.tile([128, NW], dt.bfloat16, tag="u1")
                nc.vector.scalar_tensor_tensor(
                    out=u1[:], in0=acl[:], scalar=1.0, in1=mg[:], op0=Alu.add, op1=Alu.mult,
                )
                sidx = nwp.tile([128, NW], dt.int16, tag="cnt8i")
                nc.vector.tensor_scalar(out=sidx[:], in0=u1[:], scalar1=-1.0, scalar2=None, op0=Alu.add)
                d16 = nwp.tile([128, NW], dt.int16, tag="u1")
                nc.vector.tensor_copy(d16[:, 0:1], U[:, 0:1])
                nc.vector.scalar_tensor_tensor(
                    out=d16[:, 1:NW], in0=I16[:, 0:NW - 1], scalar=256.0, in1=U[:, 1:NW],
                    op0=Alu.mult, op1=Alu.add,
                )
                W34 = sml.tile([128, 34], dt.int16, tag="W34")
                nc.gpsimd.local_scatter(W34[:], wio[:], sidx[:], channels=128, num_elems=34, num_idxs=NW)
                D34 = sml.tile([128, 34], dt.int16, tag="D34")
                nc.gpsimd.local_scatter(D34[:], d16[:], sidx[:], channels=128, num_elems=34, num_idxs=NW)
                nc.vector.tensor_tensor_scan(
                    out=Wall[:, t * K:(t + 1) * K], data0=W34[:, 0:K], data1=W34[:, 0:K],
                    initial=0.0, op0=Alu.max, op1=Alu.max,
                )
                nc.vector.tensor_tensor_scan(
                    out=Dall[:, t * K:(t + 1) * K], data0=D34[:, 0:K], data1=D34[:, 0:K],
                    initial=0.0, op0=Alu.max, op1=Alu.max,
                )

            # ---------- batched tail over all 8 tiles ----------
            DI = sml.tile([128, 256], dt.int32, tag="DI")
            nc.vector.tensor_copy(DI[:], Dall[:])
            p8k = sml.tile([128, 256], dt.int32, tag="p8k")
            nc.vector.tensor_scalar(out=p8k[:], in0=DI[:], scalar1=255, scalar2=None, op0=Alu.bitwise_and)
            ek = sml.tile([128, 256], dt.int32, tag="ek")
            nc.vector.tensor_scalar(out=ek[:], in0=DI[:], scalar1=8, scalar2=None, op0=Alu.logical_shift_right)
            # r-pre: t2 = k + 1
            t2 = sml.tile([128, 256], dt.int32, tag="t2")
            nc.vector.tensor_scalar(out=t2[:], in0=kio[:], scalar1=1, scalar2=None, op0=Alu.add)
            # batched 8-step bit-rank loop: pos = position of r-th set bit
            # of p8k where r = k + 1 - ek
            r8 = sml.tile([128, 256], dt.int32, tag="r8")
            nc.vector.tensor_tensor(out=r8[:], in0=t2[:], in1=ek[:], op=Alu.subtract)
            run = sml.tile([128, 256], dt.int32, tag="run")
            pos = sml.tile([128, 256], dt.int32, tag="pos")
            nc.vector.memset(run[:], 0)
            nc.vector.memset(pos[:], 0)
            bitb = sml.tile([128, 256], dt.int32, tag="bitb")
            cmp = sml.tile([128, 256], dt.int32, tag="cmp")
            for b in range(8):
                nc.vector.tensor_scalar(
                    out=bitb[:], in0=p8k[:], scalar1=b, scalar2=1,
                    op0=Alu.logical_shift_right, op1=Alu.bitwise_and,
                )
                nc.vector.tensor_tensor(out=run[:], in0=run[:], in1=bitb[:], op=Alu.add)
                nc.vector.tensor_tensor(out=cmp[:], in0=run[:], in1=r8[:], op=Alu.is_lt)
                nc.vector.tensor_tensor(out=pos[:], in0=pos[:], in1=cmp[:], op=Alu.add)
            posf = sml.tile([128, 256], dt.float32, tag="posf")
            nc.vector.tensor_copy(posf[:], pos[:])
            wk = sml.tile([128, 256], dt.float32, tag="wk")
            nc.vector.tensor_scalar(out=wk[:], in0=Wall[:], scalar1=-1.0, scalar2=8.0, op0=Alu.add, op1=Alu.mult)
            pk = sml.tile([128, 256], dt.float32, tag="pk")
            nc.vector.tensor_tensor(out=pk[:], in0=wk[:], in1=posf[:], op=Alu.add)
            # final select: k < ctot ? pk : (ctot>0 ? pk[tile,0] : 0)
            kiof = sml.tile([128, 256], dt.float32, tag="kiof")
            nc.vector.tensor_copy(kiof[:], kio[:])
            valid = sml.tile([128, 256], dt.float32, tag="valid")
            nc.vector.tensor_tensor(
                out=valid[:].rearrange("p (t k) -> p t k", k=K),
                in0=kiof[:].rearrange("p (t k) -> p t k", k=K),
                in1=ctall[:].unsqueeze(2).to_broadcast([128, 8, K]),
                op=Alu.is_lt,
            )
            anz = sml.tile([128, 8], dt.float32, tag="anz")
            nc.vector.tensor_scalar(out=anz[:], in0=ctall[:], scalar1=0.0, scalar2=None, op0=Alu.is_gt)
            fb = sml.tile([128, 8], dt.float32, tag="fb")
            nc.vector.tensor_tensor(
                out=fb[:], in0=pk[:].rearrange("p (t k) -> p t k", k=K)[:, :, 0],
                in1=anz[:], op=Alu.mult,
            )
            pv1 = sml.tile([128, 256], dt.float32, tag="pv1")
            nc.vector.tensor_tensor(out=pv1[:], in0=pk[:], in1=valid[:], op=Alu.mult)
            inv = sml.tile([128, 256], dt.float32, tag="inv")
            nc.vector.tensor_scalar(out=inv[:], in0=valid[:], scalar1=-1.0, scalar2=1.0, op0=Alu.mult, op1=Alu.add)
            fbb = sml.tile([128, 256], dt.float32, tag="fbb")
            nc.vector.tensor_tensor(
                out=fbb[:].rearrange("p (t k) -> p t k", k=K),
                in0=inv[:].rearrange("p (t k) -> p t k", k=K),
                in1=fb[:].unsqueeze(2).to_broadcast([128, 8, K]),
                op=Alu.mult,
            )
            idxf = sml.tile([128, 256], dt.float32, tag="idxf")
            nc.vector.tensor_tensor(out=idxf[:], in0=pv1[:], in1=fbb[:], op=Alu.add)
            if debug:
                idxi = sml.tile([128, 256], dt.int32, tag="idxi")
                nc.vector.tensor_copy(idxi[:], idxf[:])
                nc.sync.dma_start(
                    dbg["idx"].rearrange("(t j) k -> j t k", t=NT), idxi[:].rearrange("p (t k) -> p t k", k=K)
                )
            nc.vector.tensor_copy(i16o[:], idxf[:])
            # wrap: global flat f = (t*128+j)*32+k -> wrp[q = k%16, 2*(t*128+j) + k//16]
            tra = sml.tile([128, 128], dt.int16, tag="tra")
            trb = sml.tile([128, 128], dt.int16, tag="trb")
            nc.sync.dma_start_transpose(tra[:], i16o[:, 0:128])
            nc.sync.dma_start_transpose(trb[:], i16o[:, 128:256])
            # tr row (t*32 + kh*16 + q) col j  ->  wrp[q, t*256 + 2j + kh]
            for t in range(NT):
                tr = tra if t < 4 else trb
                base = (t % 4) * 32
                for kh in range(2):
                    nc.sync.dma_start(
                        wrp[:, t * 256:(t + 1) * 256].rearrange("p (j h) -> p j h", h=2)[:, :, kh:kh + 1],
                        tr[base + kh * 16:base + kh * 16 + 16, :].unsqueeze(2),
                    )
            for gg in range(8):
                nc.sync.dma_start(wrep[gg * 16:(gg + 1) * 16, :], wrp[:])
            # gather + coord subtract + out, per tile halves (64 centers)
            for t in range(NT):
                for hh in range(2):
                    j0 = t * 128 + hh * 64
                    gbuf = gat.tile([128, K * 64], dt.float32, tag="gbuf")
                    nc.gpsimd.ap_gather(
                        out_ap=gbuf[:].unsqueeze(2),
                        in_ap=bigbuf[:].unsqueeze(2),
                        idxs_ap=wrep[:, t * 256 + hh * 128:t * 256 + (hh + 1) * 128],
                        channels=128, num_elems=N, d=1, num_idxs=K * 64,
                    )
                    cslice = crow[64:67, j0:j0 + 64]
                    nc.vector.tensor_tensor(
                        out=gbuf[64:67, :].rearrange("d (j k) -> d j k", k=K),
                        in0=gbuf[64:67, :].rearrange("d (j k) -> d j k", k=K),
                        in1=cslice.unsqueeze(2).to_broadcast([3, 64, K]),
                        op=Alu.subtract,
                    )
                    nc.sync.dma_start(
                        out[3:3 + C, j0:j0 + 64, :], gbuf[0:64, :].rearrange("c (j k) -> c j k", k=K)
                    )
                    nc.sync.dma_start(
                        out[0:3, j0:j0 + 64, :], gbuf[64:67, :].rearrange("c (j k) -> c j k", k=K)
                    )
        pers_cm.__exit__(None, None, None)
    nc.compile()
    return nc


# ----------------------------------------------------------------------------
# Harness entry point: FULL inputs -> FULL output, sharded over 8 NeuronCores.
# Core c handles batch c//2, centers half c%2 (data-parallel B x M-shard).
# ----------------------------------------------------------------------------
_NC_CACHE = {}


def kernel(points_coords, centers_coords, points_features):
    import numpy as np
    from concourse.bass_utils import run_bass_kernel_spmd

    if "nc" not in _NC_CACHE:
        _NC_CACHE["nc"] = build(debug=False)
    nc = _NC_CACHE["nc"]

    pts = np.ascontiguousarray(np.asarray(points_coords, dtype=np.float32))
    ctr = np.ascontiguousarray(np.asarray(centers_coords, dtype=np.float32))
    feat = np.ascontiguousarray(np.asarray(points_features, dtype=np.float32))
    B, _, n = pts.shape
    M = ctr.shape[2]
    consts = host_consts()
    in_maps = []
    for core in range(8):
        b, h = core // 2, core % 2
        in_maps.append({
            "pts": pts[b],
            "ctr": np.ascontiguousarray(ctr[b, :, h * MH:(h + 1) * MH]),
            "feat": feat[b],
            **consts,
        })
    res = run_bass_kernel_spmd(nc, in_maps, core_ids=list(range(8)))
    outf = np.zeros((B, 3 + C, M, K), dtype=np.float32)
    for core in range(8):
        b, h = core // 2, core % 2
        outf[b][:, h * MH:(h + 1) * MH] = res.results[core]["out"]
    return outf
